# revision 16
# baseline (speedup 1.0000x reference)
"""BiESN2D on 8 TRN2 NeuronCores (Bass/Tile) — z-form recurrence.

Reference computes 4 directional leaky-tanh ESN scans over a (8,128,128,64)
image batch: horizontal fwd/bwd over rows, vertical fwd/bwd over columns,
each with U=256 units, outputs concatenated to (8,128,128,1024).

Sharding: core = (scan-direction, batch-half).  Each of the 8 cores runs ONE
scan type over S=512 sequences (4 batches x 128 rows/cols), T=128 steps.

Device recurrence is kept in PRE-ACTIVATION form (z_t), which removes the
VectorE blend from the serial chain entirely:
    z_{t+1} = (x_{t+1} - 0.1 x_t) @ Wk  +  0.1 z_t  +  tanh(z_t) @ Wr'
with Wr' = 0.9 Wr.  The three terms map to hardware as:
  - x-term: ONE K=128 matmul per u'-tile; the moving operand packs x_t in
    partitions 0:64 and x_{t-1} in 64:128, the stationary packs [Wk; -0.1*Wk].
  - 0.1 z_t: a whole window of W=4 steps accumulates in ONE psum bank with
    step-p weights pre-scaled by 10^p, so the decayed carry is plain PSUM
    accumulation (A_p = z_{b+p} * 10^p); tanh reads with scale=10^-p (free
    immediate on ScalarE).  At window boundaries DVE copies A * 10^-W to an
    SBUF fp16 scratch (in parallel with tanh), and an identity matmul
    opens the next bank with it (start=True on its first M-tile).
  - tanh term: 4 K=128 matmuls per chunk reading the previous g from an
    SBUF ring (fp16).
Per chunk-step the serial chain is just: matmuls -> tanh -> matmuls.
The device emits raw g_t = tanh(z_t); the host reconstructs
h_t = 0.9 * sum_k 0.1^k g_{t-k} (5 terms, error ~1e-5) after the gather.

Structure per step, per s-chunk (3 independent recurrence chains):
  - 2 x-matmuls + 4 recurrent matmuls accumulate into the window's psum
    bank (start=False across steps; skip_group_check since the sim's group
    tracker doesn't model cross-step accumulation, which is legal on HW),
  - one ScalarE tanh (psum fp32 -> ring slot fp16, scale=10^-p),
  - every 4 steps, one batched DMA of 4 ring slots to DRAM.
PSUM is written ONLY by matmuls: an earlier variant seeded window banks
with a DVE tensor_scalar psum->psum write and the first boundary then
intermittently read a partially-visible bank (cold-run races, ~3e-1
absmax).  Routing the seed through SBUF + identity matmul (the bank
opener uses the start=True-on-first-M-tile pattern of the reference
kernel) removes all non-matmul psum writes.
All compute except PSUM accumulation is fp16.
"""

import numpy as np
from contextlib import ExitStack

import concourse.bass as bass
import concourse.mybir as mybir
import concourse.tile as tile
from concourse import bacc
from concourse.bass_utils import run_bass_kernel_spmd

# ---------------- problem constants (hardcoded per spec) ----------------
B, NH, NW, C = 8, 128, 128, 64
U = 256           # units per directional ESN cell
T = 128           # scan length
S = 512           # sequences per core (4 batches * 128)
LEAKY = 0.9
N_CORES = 8

F16 = mybir.dt.float16
F32 = mybir.dt.float32

CHUNKS = (176, 176, 160)  # s-chunks; each <= 256 (two u'-tiles in one bank)
W = 4                     # steps per psum accumulation window (scales 10^p)
RING = 20                 # g-ring slots per chain (rides out the x-in burst)
DMA_BATCH = 4             # t-steps per output DMA
XDMA_TGROUP = 16          # t-steps per input DMA chunk
XDMA_FIRST = 4            # smaller first x group: compute starts sooner
HOST_TERMS = 5            # 0.1^k series terms for host-side h reconstruction


def build_program(chunks=CHUNKS, t_steps=T, s_total=S, win=W):
    """Build the SPMD per-core Bass program (identical on all 8 cores)."""
    assert sum(chunks) == s_total and all(c <= 256 for c in chunks)
    assert t_steps % DMA_BATCH == 0 and RING % DMA_BATCH == 0

    nc = bacc.Bacc("TRN2", target_bir_lowering=False, debug=False,
                   num_devices=N_CORES)

    # x packed two time steps deep:
    #   x_d[c,      t*S + s] = x[s, t,   c]
    #   x_d[64 + c, t*S + s] = x[s, t-1, c]   (zeros for t == 0)
    x_d = nc.declare_dram_parameter("x", [128, t_steps * s_total], F16,
                                    isOutput=False)
    # per window position p: wk[:, p*256:(p+1)*256] = [Wk; -0.1*Wk] * 10^p
    wk_d = nc.declare_dram_parameter("wk", [128, win * 256], F16,
                                     isOutput=False)
    # wr{0,1}[:, p*256+j*128 : ...] = (0.9*Wr*10^p)[k-half, j*128:(j+1)*128]
    wr0_d = nc.declare_dram_parameter("wr0", [128, win * 256], F16,
                                      isOutput=False)
    wr1_d = nc.declare_dram_parameter("wr1", [128, win * 256], F16,
                                      isOutput=False)
    eye_d = nc.declare_dram_parameter("eye", [128, 128], F16, isOutput=False)
    # per-chain outputs: y{ch}[p, t, j, s_local] = g_t[u = j*128 + p, s]
    y_aps = [nc.declare_dram_parameter(f"y{ch}", [128, t_steps, 2, ncs],
                                       F16, isOutput=True).ap()
             for ch, ncs in enumerate(chunks)]
    x_ap, wk_ap = x_d.ap(), wk_d.ap()
    wr0_ap, wr1_ap, eye_ap = wr0_d.ap(), wr1_d.ap(), eye_d.ap()

    nch = len(chunks)
    offs = [sum(chunks[:i]) for i in range(nch)]
    Tanh = mybir.ActivationFunctionType.Tanh

    with ExitStack() as ctx:
        tc = ctx.enter_context(tile.TileContext(nc))
        const = ctx.enter_context(tc.tile_pool(name="const", bufs=1))
        x_sb = const.tile([128, t_steps * s_total], F16)
        wk_sb = const.tile([128, win * 256], F16)
        wr0_sb = const.tile([128, win * 256], F16)
        wr1_sb = const.tile([128, win * 256], F16)
        eye_sb = const.tile([128, 128], F16)
        # per-chain boundary-seed scratch (fp16 copy of A * 10^-W)
        zcs = [const.tile([128, 2 * chunks[ch]], F16, name=f"zc{ch}")
               for ch in range(nch)]
        # per-chain g rings: slot k at cols [k*2*ncs, (k+1)*2*ncs)
        rings = [const.tile([128, RING * 2 * chunks[ch]], F16,
                            name=f"gring{ch}") for ch in range(nch)]

        nc.sync.dma_start(wk_sb[:], wk_ap[:])
        nc.sync.dma_start(wr0_sb[:], wr0_ap[:])
        nc.sync.dma_start(wr1_sb[:], wr1_ap[:])
        nc.sync.dma_start(eye_sb[:], eye_ap[:])
        # x input: everything stays on the sync HWDGE queue (scalar-queue
        # and SWDGE experiments both produced first-window races / stalls),
        # but only the first few groups are enqueued up front.  Later groups
        # are emitted from inside the t-loop, so the per-4-step output DMAs
        # interleave with them in queue order instead of waiting behind the
        # whole 16MB burst (which stalled the pipeline when the g-ring
        # filled, and HAM down-clocked during the stall).  The small first
        # groups also pace the first steps the way the (race-free) v2
        # schedule did, instead of compressing them against t=0.
        xbounds = [0, XDMA_FIRST]
        while xbounds[-1] < t_steps:
            xbounds.append(min(xbounds[-1] + XDMA_TGROUP, t_steps))
        xdma = [(glo * s_total, ghi * s_total)
                for glo, ghi in zip(xbounds[:-1], xbounds[1:])]
        n_upfront = 3                       # covers t < 36
        for lo, hi in xdma[:n_upfront]:
            nc.sync.dma_start(x_sb[:, lo:hi], x_ap[:, lo:hi])
        # lazy group k emitted at step xbounds[k] - 16 (one group of lead)
        lazy_at = {max(0, xbounds[k] - XDMA_TGROUP): k
                   for k in range(n_upfront, len(xdma))}

        ps_pool = ctx.enter_context(tc.tile_pool(name="ps", bufs=1,
                                                 space="PSUM"))
        # two full psum banks per chain, ping-ponged per window
        pbanks = [[ps_pool.tile([128, 512], F32, tag=f"pb{ch}_{b}",
                                name=f"pb{ch}_{b}") for b in range(2)]
                  for ch in range(nch)]

        def gslot(ch, t):
            ncs = chunks[ch]
            k = t % RING
            return rings[ch][:, k * 2 * ncs:(k + 1) * 2 * ncs]

        for t in range(t_steps):
            if t in lazy_at:
                lo, hi = xdma[lazy_at[t]]
                nc.sync.dma_start(x_sb[:, lo:hi], x_ap[:, lo:hi])
            w_i, p = divmod(t, win)
            bank = w_i % 2
            scl = 256 * p
            for ch in range(nch):
                ncs, off = chunks[ch], offs[ch]
                ps = pbanks[ch][bank][:, 0:2 * ncs]
                if p == 0 and t > 0:
                    # seed next window's bank: 0.1 * z = A_{W-1} * 10^-W.
                    # DVE reads psum but writes SBUF fp16 (never psum);
                    # identity matmuls then open the bank with the seed.
                    nc.vector.tensor_scalar_mul(
                        zcs[ch][:], pbanks[ch][1 - bank][:, 0:2 * ncs],
                        10.0 ** -win)
                    for j in range(2):
                        nc.tensor.matmul(ps[:, j * ncs:(j + 1) * ncs],
                                         eye_sb[:],
                                         zcs[ch][:, j * ncs:(j + 1) * ncs],
                                         start=(j == 0), stop=False,
                                         skip_group_check=True)
                xcol = t * s_total + off
                for j in range(2):
                    nc.tensor.matmul(ps[:, j * ncs:(j + 1) * ncs],
                                     wk_sb[:, scl + j * 128:scl + (j + 1) * 128],
                                     x_sb[:, xcol:xcol + ncs],
                                     start=(t == 0 and j == 0), stop=False,
                                     skip_group_check=True)
                if t > 0:
                    gp = gslot(ch, t - 1)
                    nc.tensor.matmul(ps[:, 0:ncs],
                                     wr0_sb[:, scl:scl + 128],
                                     gp[:, 0:ncs], start=False, stop=False,
                                     skip_group_check=True)
                    nc.tensor.matmul(ps[:, 0:ncs],
                                     wr1_sb[:, scl:scl + 128],
                                     gp[:, ncs:2 * ncs], start=False,
                                     stop=False, skip_group_check=True)
                    nc.tensor.matmul(ps[:, ncs:2 * ncs],
                                     wr0_sb[:, scl + 128:scl + 256],
                                     gp[:, 0:ncs], start=False, stop=False,
                                     skip_group_check=True)
                    nc.tensor.matmul(ps[:, ncs:2 * ncs],
                                     wr1_sb[:, scl + 128:scl + 256],
                                     gp[:, ncs:2 * ncs], start=False,
                                     stop=True, skip_group_check=True)

                g = gslot(ch, t)
                nc.scalar.activation(g[:], ps, Tanh, scale=10.0 ** -p)

                if t % DMA_BATCH == DMA_BATCH - 1:
                    # ring slots for [t-3 .. t] are contiguous
                    k0 = (t - (DMA_BATCH - 1)) % RING
                    src = rings[ch][:, k0 * 2 * ncs:
                                    (k0 + DMA_BATCH) * 2 * ncs]
                    dst = y_aps[ch][:, t - (DMA_BATCH - 1):t + 1, :, :]
                    nc.sync.dma_start(dst, src)

    nc.compile()
    return nc


_PROGRAM = None

# test-harness knob: when trace=True, the BassKernelResults (with
# exec_time_ns from neuron-profile) is stashed in PROFILE["last"].
PROFILE = {"trace": False, "last": None}


def _get_program():
    global _PROGRAM
    if _PROGRAM is None:
        _PROGRAM = build_program()
    return _PROGRAM


def _pack_x(xs, t_steps, s_total):
    """(S, T, C) fp32 -> packed (128, T*S) fp16: rows 0:64 hold x_t,
    rows 64:128 hold x_{t-1} (zeros at t=0)."""
    xt = np.ascontiguousarray(xs.transpose(2, 1, 0))      # (C, T, S)
    packed = np.empty((128, t_steps * s_total), np.float16)
    pv = packed.reshape(2, 64, t_steps * s_total)
    pv[0] = xt.reshape(64, -1)
    pv[1, :, s_total:] = pv[0][:, :-s_total]
    pv[1, :, :s_total] = 0.0
    return packed


def _pack_weights(wk, wr):
    """Window-scaled weight panels for one scan direction."""
    wk = np.asarray(wk, np.float32)                        # (64, 256)
    wrp = LEAKY * np.asarray(wr, np.float32)               # (256, 256)
    wk_all = np.empty((128, W * 256), np.float16)
    wr0_all = np.empty((128, W * 256), np.float16)
    wr1_all = np.empty((128, W * 256), np.float16)
    for p in range(W):
        f = 10.0 ** p
        blk = slice(p * 256, (p + 1) * 256)
        wk_all[0:64, blk] = (f * wk).astype(np.float16)
        wk_all[64:128, blk] = (-0.1 * f * wk).astype(np.float16)
        wr0_all[:, blk] = (f * wrp[0:128]).astype(np.float16)
        wr1_all[:, blk] = (f * wrp[128:256]).astype(np.float16)
    return wk_all, wr0_all, wr1_all


def kernel(**inputs):
    x = np.asarray(inputs["inputs"], np.float32)          # (8,128,128,64)
    wsets = [
        (inputs["h_fwd_k"], inputs["h_fwd_r"]),
        (inputs["h_bwd_k"], inputs["h_bwd_r"]),
        (inputs["v_fwd_k"], inputs["v_fwd_r"]),
        (inputs["v_bwd_k"], inputs["v_bwd_r"]),
    ]
    nc = _get_program()

    in_maps = []
    for core in range(N_CORES):
        scan, bhalf = core // 2, core % 2
        xb = x[bhalf * 4:(bhalf + 1) * 4]                 # (4, NH, NW, C)
        if scan >= 2:                                     # vertical: cols as seqs
            xb = xb.transpose(0, 2, 1, 3)                 # (4, NW, NH, C)
        xs = xb.reshape(S, T, C)
        if scan % 2 == 1:                                 # bwd: reverse time
            xs = np.ascontiguousarray(xs[:, ::-1])
        wk_all, wr0_all, wr1_all = _pack_weights(*wsets[scan])
        in_maps.append({"x": _pack_x(xs, T, S), "wk": wk_all,
                        "wr0": wr0_all, "wr1": wr1_all,
                        "eye": np.eye(128, dtype=np.float16)})

    res = run_bass_kernel_spmd(nc, in_maps, list(range(N_CORES)),
                               trace=PROFILE["trace"])
    PROFILE["last"] = res
    results = res.results

    out = np.empty((B, NH, NW, 4 * U), np.float32)
    for core in range(N_CORES):
        scan, bhalf = core // 2, core % 2
        # concat per-chain outputs (128, T, 2, ncs) back to (128, T, 2, S)
        g = np.concatenate([results[core][f"y{ch}"]
                            for ch in range(len(CHUNKS))],
                           axis=3).astype(np.float32)
        # h_t = 0.9 * sum_k 0.1^k g_{t-k}  (device time order)
        acc = g.copy()
        f = 1.0
        for k in range(1, HOST_TERMS):
            f *= 1.0 - LEAKY
            acc[:, k:] += f * g[:, :-k]
        h = LEAKY * acc
        hs = h.transpose(3, 1, 2, 0).reshape(S, T, U)     # (s, t, u=(j,p))
        if scan % 2 == 1:
            hs = hs[:, ::-1]
        dst = out[bhalf * 4:(bhalf + 1) * 4, :, :, scan * U:(scan + 1) * U]
        if scan < 2:
            dst[:] = hs.reshape(4, NH, NW, U)
        else:
            dst[:] = hs.reshape(4, NW, NH, U).transpose(0, 2, 1, 3)
    return out


# revision 17
# speedup vs baseline: 1.0395x; 1.0395x over previous
"""BiESN2D on 8 TRN2 NeuronCores (Bass/Tile) — z-form recurrence.

Reference computes 4 directional leaky-tanh ESN scans over a (8,128,128,64)
image batch: horizontal fwd/bwd over rows, vertical fwd/bwd over columns,
each with U=256 units, outputs concatenated to (8,128,128,1024).

Sharding: core = (scan-direction, batch-half).  Each of the 8 cores runs ONE
scan type over S=512 sequences (4 batches x 128 rows/cols), T=128 steps.

Device recurrence is kept in PRE-ACTIVATION form (z_t), which removes the
VectorE blend from the serial chain entirely:
    z_{t+1} = (x_{t+1} - 0.1 x_t) @ Wk  +  0.1 z_t  +  tanh(z_t) @ Wr'
with Wr' = 0.9 Wr.  The three terms map to hardware as:
  - x-term: ONE K=128 matmul per u'-tile; the moving operand packs x_t in
    partitions 0:64 and x_{t-1} in 64:128, the stationary packs [Wk; -0.1*Wk].
  - 0.1 z_t: a whole window of W=4 steps accumulates in ONE psum bank with
    step-p weights pre-scaled by 10^p, so the decayed carry is plain PSUM
    accumulation (A_p = z_{b+p} * 10^p); tanh reads with scale=10^-p (free
    immediate on ScalarE).  At window boundaries DVE copies A * 10^-W to an
    SBUF fp16 scratch (in parallel with tanh), and an identity matmul
    opens the next bank with it (start=True on its first M-tile).
  - tanh term: 4 K=128 matmuls per chunk reading the previous g from an
    SBUF ring (fp16).
Per chunk-step the serial chain is just: matmuls -> tanh -> matmuls.
The device emits raw g_t = tanh(z_t); the host reconstructs
h_t = 0.9 * sum_k 0.1^k g_{t-k} (5 terms, error ~1e-5) after the gather.

Structure per step, per s-chunk (3 independent recurrence chains):
  - 2 x-matmuls + 4 recurrent matmuls accumulate into the window's psum
    bank (start=False across steps; skip_group_check since the sim's group
    tracker doesn't model cross-step accumulation, which is legal on HW),
  - one ScalarE tanh (psum fp32 -> ring slot fp16, scale=10^-p),
  - every 4 steps, one batched DMA of 4 ring slots to DRAM.
PSUM is written ONLY by matmuls: an earlier variant seeded window banks
with a DVE tensor_scalar psum->psum write and the first boundary then
intermittently read a partially-visible bank (cold-run races, ~3e-1
absmax).  Routing the seed through SBUF + identity matmul (the bank
opener uses the start=True-on-first-M-tile pattern of the reference
kernel) removes all non-matmul psum writes.
All compute except PSUM accumulation is fp16.
"""

import numpy as np
from contextlib import ExitStack

import concourse.bass as bass
import concourse.mybir as mybir
import concourse.tile as tile
from concourse import bacc
from concourse.bass_utils import run_bass_kernel_spmd

# ---------------- problem constants (hardcoded per spec) ----------------
B, NH, NW, C = 8, 128, 128, 64
U = 256           # units per directional ESN cell
T = 128           # scan length
S = 512           # sequences per core (4 batches * 128)
LEAKY = 0.9
N_CORES = 8

F16 = mybir.dt.float16
F32 = mybir.dt.float32

CHUNKS = (176, 176, 160)  # s-chunks; each <= 256 (two u'-tiles in one bank)
W = 4                     # steps per psum accumulation window (scales 10^p)
RING = 20                 # g-ring slots per chain (rides out the x-in burst)
DMA_BATCH = 4             # t-steps per output DMA
XDMA_TGROUP = 16          # t-steps per input DMA chunk
XDMA_FIRST = 4            # smaller first x group: compute starts sooner
HOST_TERMS = 5            # 0.1^k series terms for host-side h reconstruction


def build_program(chunks=CHUNKS, t_steps=T, s_total=S, win=W):
    """Build the SPMD per-core Bass program (identical on all 8 cores)."""
    assert sum(chunks) == s_total and all(c <= 256 for c in chunks)
    assert t_steps % DMA_BATCH == 0 and RING % DMA_BATCH == 0

    nc = bacc.Bacc("TRN2", target_bir_lowering=False, debug=False,
                   num_devices=N_CORES)

    # x packed two time steps deep:
    #   x_d[c,      t*S + s] = x[s, t,   c]
    #   x_d[64 + c, t*S + s] = x[s, t-1, c]   (zeros for t == 0)
    x_d = nc.declare_dram_parameter("x", [128, t_steps * s_total], F16,
                                    isOutput=False)
    # per window position p: wk[:, p*256:(p+1)*256] = [Wk; -0.1*Wk] * 10^p
    wk_d = nc.declare_dram_parameter("wk", [128, win * 256], F16,
                                     isOutput=False)
    # wr{0,1}[:, p*256+j*128 : ...] = (0.9*Wr*10^p)[k-half, j*128:(j+1)*128]
    wr0_d = nc.declare_dram_parameter("wr0", [128, win * 256], F16,
                                      isOutput=False)
    wr1_d = nc.declare_dram_parameter("wr1", [128, win * 256], F16,
                                      isOutput=False)
    eye_d = nc.declare_dram_parameter("eye", [128, 128], F16, isOutput=False)
    # per-chain outputs: y{ch}[p, t, j, s_local] = g_t[u = j*128 + p, s]
    y_aps = [nc.declare_dram_parameter(f"y{ch}", [128, t_steps, 2, ncs],
                                       F16, isOutput=True).ap()
             for ch, ncs in enumerate(chunks)]
    x_ap, wk_ap = x_d.ap(), wk_d.ap()
    wr0_ap, wr1_ap, eye_ap = wr0_d.ap(), wr1_d.ap(), eye_d.ap()

    nch = len(chunks)
    offs = [sum(chunks[:i]) for i in range(nch)]
    Tanh = mybir.ActivationFunctionType.Tanh

    with ExitStack() as ctx:
        tc = ctx.enter_context(tile.TileContext(nc))
        const = ctx.enter_context(tc.tile_pool(name="const", bufs=1))
        x_sb = const.tile([128, t_steps * s_total], F16)
        wk_sb = const.tile([128, win * 256], F16)
        wr0_sb = const.tile([128, win * 256], F16)
        wr1_sb = const.tile([128, win * 256], F16)
        eye_sb = const.tile([128, 128], F16)
        # per-chain boundary-seed scratch (fp16 copy of A * 10^-W)
        zcs = [const.tile([128, 2 * chunks[ch]], F16, name=f"zc{ch}")
               for ch in range(nch)]
        # per-chain g rings: slot k at cols [k*2*ncs, (k+1)*2*ncs)
        rings = [const.tile([128, RING * 2 * chunks[ch]], F16,
                            name=f"gring{ch}") for ch in range(nch)]

        nc.sync.dma_start(wk_sb[:], wk_ap[:])
        nc.sync.dma_start(wr0_sb[:], wr0_ap[:])
        nc.sync.dma_start(wr1_sb[:], wr1_ap[:])
        nc.sync.dma_start(eye_sb[:], eye_ap[:])
        # x input: everything stays on the sync HWDGE queue (scalar-queue
        # and SWDGE experiments both produced first-window races / stalls),
        # but only the first few groups are enqueued up front.  Later groups
        # are emitted from inside the t-loop, so the per-4-step output DMAs
        # interleave with them in queue order instead of waiting behind the
        # whole 16MB burst (which stalled the pipeline when the g-ring
        # filled, and HAM down-clocked during the stall).  The small first
        # groups also pace the first steps the way the (race-free) v2
        # schedule did, instead of compressing them against t=0.
        xbounds = [0, XDMA_FIRST]
        while xbounds[-1] < t_steps:
            xbounds.append(min(xbounds[-1] + XDMA_TGROUP, t_steps))
        xdma = [(glo * s_total, ghi * s_total)
                for glo, ghi in zip(xbounds[:-1], xbounds[1:])]
        n_upfront = 3                       # covers t < 36
        for lo, hi in xdma[:n_upfront]:
            nc.sync.dma_start(x_sb[:, lo:hi], x_ap[:, lo:hi])
        # lazy group k emitted at step xbounds[k] - 16 (one group of lead)
        lazy_at = {max(0, xbounds[k] - XDMA_TGROUP): k
                   for k in range(n_upfront, len(xdma))}

        ps_pool = ctx.enter_context(tc.tile_pool(name="ps", bufs=1,
                                                 space="PSUM"))
        # two full psum banks per chain, ping-ponged per window
        pbanks = [[ps_pool.tile([128, 512], F32, tag=f"pb{ch}_{b}",
                                name=f"pb{ch}_{b}") for b in range(2)]
                  for ch in range(nch)]

        def gslot(ch, t):
            ncs = chunks[ch]
            k = t % RING
            return rings[ch][:, k * 2 * ncs:(k + 1) * 2 * ncs]

        for t in range(t_steps):
            if t in lazy_at:
                lo, hi = xdma[lazy_at[t]]
                nc.sync.dma_start(x_sb[:, lo:hi], x_ap[:, lo:hi])
            w_i, p = divmod(t, win)
            bank = w_i % 2
            scl = 256 * p
            for ch in range(nch):
                ncs, off = chunks[ch], offs[ch]
                ps = pbanks[ch][bank][:, 0:2 * ncs]
                boundary = p == 0 and t > 0
                if boundary:
                    # seed next window's bank: 0.1 * z = A_{W-1} * 10^-W.
                    # DVE reads psum but writes SBUF fp16 (never psum).
                    nc.vector.tensor_scalar_mul(
                        zcs[ch][:], pbanks[ch][1 - bank][:, 0:2 * ncs],
                        10.0 ** -win)
                # The x matmuls OPEN every window's bank (start=True on the
                # first M-tile): they depend only on x, so the scheduler can
                # run them under the previous step's tanh instead of behind
                # the seed copy, keeping the boundary chain near the normal
                # step's length.  The identity-seed matmuls then accumulate.
                xcol = t * s_total + off
                for j in range(2):
                    nc.tensor.matmul(ps[:, j * ncs:(j + 1) * ncs],
                                     wk_sb[:, scl + j * 128:scl + (j + 1) * 128],
                                     x_sb[:, xcol:xcol + ncs],
                                     start=((t == 0 or boundary) and j == 0),
                                     stop=False, skip_group_check=True)
                if boundary:
                    for j in range(2):
                        nc.tensor.matmul(ps[:, j * ncs:(j + 1) * ncs],
                                         eye_sb[:],
                                         zcs[ch][:, j * ncs:(j + 1) * ncs],
                                         start=False, stop=False,
                                         skip_group_check=True)
                if t > 0:
                    gp = gslot(ch, t - 1)
                    nc.tensor.matmul(ps[:, 0:ncs],
                                     wr0_sb[:, scl:scl + 128],
                                     gp[:, 0:ncs], start=False, stop=False,
                                     skip_group_check=True)
                    nc.tensor.matmul(ps[:, 0:ncs],
                                     wr1_sb[:, scl:scl + 128],
                                     gp[:, ncs:2 * ncs], start=False,
                                     stop=False, skip_group_check=True)
                    nc.tensor.matmul(ps[:, ncs:2 * ncs],
                                     wr0_sb[:, scl + 128:scl + 256],
                                     gp[:, 0:ncs], start=False, stop=False,
                                     skip_group_check=True)
                    nc.tensor.matmul(ps[:, ncs:2 * ncs],
                                     wr1_sb[:, scl + 128:scl + 256],
                                     gp[:, ncs:2 * ncs], start=False,
                                     stop=True, skip_group_check=True)

                g = gslot(ch, t)
                nc.scalar.activation(g[:], ps, Tanh, scale=10.0 ** -p)

                if t % DMA_BATCH == DMA_BATCH - 1:
                    # ring slots for [t-3 .. t] are contiguous
                    k0 = (t - (DMA_BATCH - 1)) % RING
                    src = rings[ch][:, k0 * 2 * ncs:
                                    (k0 + DMA_BATCH) * 2 * ncs]
                    dst = y_aps[ch][:, t - (DMA_BATCH - 1):t + 1, :, :]
                    nc.sync.dma_start(dst, src)

    nc.compile()
    return nc


_PROGRAM = None

# test-harness knob: when trace=True, the BassKernelResults (with
# exec_time_ns from neuron-profile) is stashed in PROFILE["last"].
PROFILE = {"trace": False, "last": None}


def _get_program():
    global _PROGRAM
    if _PROGRAM is None:
        _PROGRAM = build_program()
    return _PROGRAM


def _pack_x(xs, t_steps, s_total):
    """(S, T, C) fp32 -> packed (128, T*S) fp16: rows 0:64 hold x_t,
    rows 64:128 hold x_{t-1} (zeros at t=0)."""
    xt = np.ascontiguousarray(xs.transpose(2, 1, 0))      # (C, T, S)
    packed = np.empty((128, t_steps * s_total), np.float16)
    pv = packed.reshape(2, 64, t_steps * s_total)
    pv[0] = xt.reshape(64, -1)
    pv[1, :, s_total:] = pv[0][:, :-s_total]
    pv[1, :, :s_total] = 0.0
    return packed


def _pack_weights(wk, wr):
    """Window-scaled weight panels for one scan direction."""
    wk = np.asarray(wk, np.float32)                        # (64, 256)
    wrp = LEAKY * np.asarray(wr, np.float32)               # (256, 256)
    wk_all = np.empty((128, W * 256), np.float16)
    wr0_all = np.empty((128, W * 256), np.float16)
    wr1_all = np.empty((128, W * 256), np.float16)
    for p in range(W):
        f = 10.0 ** p
        blk = slice(p * 256, (p + 1) * 256)
        wk_all[0:64, blk] = (f * wk).astype(np.float16)
        wk_all[64:128, blk] = (-0.1 * f * wk).astype(np.float16)
        wr0_all[:, blk] = (f * wrp[0:128]).astype(np.float16)
        wr1_all[:, blk] = (f * wrp[128:256]).astype(np.float16)
    return wk_all, wr0_all, wr1_all


def kernel(**inputs):
    x = np.asarray(inputs["inputs"], np.float32)          # (8,128,128,64)
    wsets = [
        (inputs["h_fwd_k"], inputs["h_fwd_r"]),
        (inputs["h_bwd_k"], inputs["h_bwd_r"]),
        (inputs["v_fwd_k"], inputs["v_fwd_r"]),
        (inputs["v_bwd_k"], inputs["v_bwd_r"]),
    ]
    nc = _get_program()

    in_maps = []
    for core in range(N_CORES):
        scan, bhalf = core // 2, core % 2
        xb = x[bhalf * 4:(bhalf + 1) * 4]                 # (4, NH, NW, C)
        if scan >= 2:                                     # vertical: cols as seqs
            xb = xb.transpose(0, 2, 1, 3)                 # (4, NW, NH, C)
        xs = xb.reshape(S, T, C)
        if scan % 2 == 1:                                 # bwd: reverse time
            xs = np.ascontiguousarray(xs[:, ::-1])
        wk_all, wr0_all, wr1_all = _pack_weights(*wsets[scan])
        in_maps.append({"x": _pack_x(xs, T, S), "wk": wk_all,
                        "wr0": wr0_all, "wr1": wr1_all,
                        "eye": np.eye(128, dtype=np.float16)})

    res = run_bass_kernel_spmd(nc, in_maps, list(range(N_CORES)),
                               trace=PROFILE["trace"])
    PROFILE["last"] = res
    results = res.results

    out = np.empty((B, NH, NW, 4 * U), np.float32)
    for core in range(N_CORES):
        scan, bhalf = core // 2, core % 2
        # concat per-chain outputs (128, T, 2, ncs) back to (128, T, 2, S)
        g = np.concatenate([results[core][f"y{ch}"]
                            for ch in range(len(CHUNKS))],
                           axis=3).astype(np.float32)
        # h_t = 0.9 * sum_k 0.1^k g_{t-k}  (device time order)
        acc = g.copy()
        f = 1.0
        for k in range(1, HOST_TERMS):
            f *= 1.0 - LEAKY
            acc[:, k:] += f * g[:, :-k]
        h = LEAKY * acc
        hs = h.transpose(3, 1, 2, 0).reshape(S, T, U)     # (s, t, u=(j,p))
        if scan % 2 == 1:
            hs = hs[:, ::-1]
        dst = out[bhalf * 4:(bhalf + 1) * 4, :, :, scan * U:(scan + 1) * U]
        if scan < 2:
            dst[:] = hs.reshape(4, NH, NW, U)
        else:
            dst[:] = hs.reshape(4, NW, NH, U).transpose(0, 2, 1, 3)
    return out
